# revision 1
# baseline (speedup 1.0000x reference)
"""Batched normalized-gram kernel for 8 TRN2 NeuronCores.

reference:  x (64, 2, 512, 512) fp32
    x0 = x[:, 0]                               (B=64, V=512, F=512)
    n  = sqrt(sum(x0^2, axis=(0, 2)))          (V,)
    out[b] = (x0[b] @ x0[b].T) / outer(n, n)   (B, V, V)

Since gram[b,i,j]/(n_i n_j) == (x0[b,i,:]/n_i) . (x0[b,j,:]/n_j), the host
prescales rows by 1/n once and the device work is a pure batched symmetric
matmul out[b] = y[b] @ y[b].T.

Device-side tricks:
  * operands shipped as fp16 (|y| <= ~0.05, comfortably normal) — halves
    input DMA, full-rate PE, fp32 PSUM accumulation keeps rel err ~2e-4.
  * out[b] is symmetric, and the reference is *exactly* symmetric (same
    products, same summation order), so the device computes only the upper
    block-triangle (row-block mi covers columns mi*128..511) and the host
    mirrors the lower blocks.  -37.5% output DMA, -37.5% PE work.

Sharding: data-parallel over batch — 8 batches per core, no collectives.
"""

import numpy as np

B, T, V, F = 64, 2, 512, 512
NCORES = 8
BPC = B // NCORES  # batches per core
NBLK = V // 128  # 4 row-blocks

_NC = None


def _build_nc():
    import concourse.mybir as mybir
    import concourse.tile as tile
    from concourse import bacc

    f32 = mybir.dt.float32
    f16 = mybir.dt.float16

    nc = bacc.Bacc(target_bir_lowering=False)
    yT = nc.declare_dram_parameter("yT", [BPC, F, V], f16, isOutput=False)
    out = nc.declare_dram_parameter("out", [BPC, V, V], f32, isOutput=True)

    # upper-triangle segment offsets inside the per-batch output tile:
    # row-block mi holds columns mi*128..511 (N = 512 - 128*mi)
    seg_off = [0]
    for mi in range(NBLK):
        seg_off.append(seg_off[-1] + V - 128 * mi)
    seg_total = seg_off[-1]  # 1280

    with tile.TileContext(nc) as tc:
        with (
            tc.tile_pool(name="inp", bufs=10) as inp_pool,
            tc.tile_pool(name="psum", bufs=8, space="PSUM") as psum_pool,
            tc.tile_pool(name="outp", bufs=3) as out_pool,
        ):
            for b in range(BPC):
                # yT[b] is (F, V): four partition-chunks of [128, V], one
                # tile each so matmuls depend only on the chunk they read.
                # Input DMAs ride the SP HWDGE ring; output DMAs ride the
                # ACT ring — two independent FIFOs running concurrently.
                chunks = []
                for ki in range(NBLK):
                    ck = inp_pool.tile([128, V], f16, tag="ck")
                    nc.sync.dma_start(
                        out=ck, in_=yT[b, ki * 128 : (ki + 1) * 128, :]
                    )
                    chunks.append(ck)
                ot = out_pool.tile([128, seg_total], f32)
                for mi in range(NBLK):
                    n_cols = V - 128 * mi
                    ps = psum_pool.tile([128, n_cols], f32, tag="ps")
                    for ki in range(NBLK):
                        nc.tensor.matmul(
                            ps,
                            lhsT=chunks[ki][:, mi * 128 : (mi + 1) * 128],
                            rhs=chunks[ki][:, mi * 128 :],
                            start=(ki == 0),
                            stop=(ki == NBLK - 1),
                        )
                    seg = ot[:, seg_off[mi] : seg_off[mi] + n_cols]
                    nc.vector.tensor_copy(out=seg, in_=ps)
                    nc.scalar.dma_start(
                        out=out[b, mi * 128 : (mi + 1) * 128, mi * 128 :],
                        in_=seg,
                    )
    if not nc.is_finalized():
        nc.finalize()
    return nc


def _get_nc():
    global _NC
    if _NC is None:
        _NC = _build_nc()
    return _NC


def _prep_shards(x: np.ndarray) -> np.ndarray:
    x = np.ascontiguousarray(np.asarray(x, dtype=np.float32))
    x0 = x[:, 0]  # (B, V, F)
    ss = np.einsum("bvf,bvf->v", x0, x0, optimize=True)
    inv_n = (1.0 / np.sqrt(ss)).astype(np.float32)
    y = x0 * inv_n[None, :, None]
    # (B, F, V) contiguous so each core's operand streams with unit stride
    return np.ascontiguousarray(np.transpose(y, (0, 2, 1)).astype(np.float16))


def kernel(x: np.ndarray, _trace: bool = False, _trace_out: list | None = None):
    from concourse.bass_utils import run_bass_kernel_spmd

    yT = _prep_shards(x)
    nc = _get_nc()
    in_maps = [{"yT": yT[c * BPC : (c + 1) * BPC]} for c in range(NCORES)]
    res = run_bass_kernel_spmd(
        nc, in_maps, core_ids=list(range(NCORES)), trace=_trace
    )
    if _trace_out is not None:
        _trace_out.append(res)
    full = np.concatenate(
        [np.asarray(res.results[c]["out"]) for c in range(NCORES)], axis=0
    )
    # device wrote only the upper block-triangle; mirror it down
    for mi in range(NBLK):
        for nj in range(mi + 1, NBLK):
            full[:, nj * 128 : (nj + 1) * 128, mi * 128 : (mi + 1) * 128] = (
                np.swapaxes(
                    full[:, mi * 128 : (mi + 1) * 128, nj * 128 : (nj + 1) * 128],
                    1,
                    2,
                )
            )
    return full



# revision 2
# speedup vs baseline: 1.2330x; 1.2330x over previous
"""Batched normalized-gram kernel for 8 TRN2 NeuronCores.

reference:  x (64, 2, 512, 512) fp32
    x0 = x[:, 0]                               (B=64, V=512, F=512)
    n  = sqrt(sum(x0^2, axis=(0, 2)))          (V,)
    out[b] = (x0[b] @ x0[b].T) / outer(n, n)   (B, V, V)

gram[b,i,j]/(n_i n_j) == (x0[b,i,:]/n_i) . (x0[b,j,:]/n_j), so the host
prescales rows by 1/n once and the device work is a pure batched symmetric
matmul out[b] = y[b] @ y[b].T.

Device-side structure (per core, 8 batches):
  * operands shipped as fp16 — halves input DMA, full-rate PE, fp32 PSUM
    accumulation keeps rel err ~2e-4.
  * upper block-triangle only (row-block mi covers columns mi*128..511);
    host mirrors the lower blocks.  -37.5% output DMA / PE work.
  * ONE input DMA per batch: host packs y[b].T into [128, 4*512] so the
    512 KB transfer is fully contiguous (chunk k at columns k*512..).
  * ONE output DMA per batch: the four triangle segments are packed
    [mi0|mi1|mi3|mi2] = 1280 columns so each segment's matmul stays inside
    a single 2 KB PSUM bank, and the fp16 [128, 1280] result is contiguous.
  * PSUM->SBUF copy split across Scalar (bank 0) and Vector (banks 1-2),
    which may touch PSUM concurrently on different banks.
  * a short zero-matmul warmup burst keeps the PE busy during the first
    input DMA so the HAM clock-gate reaches full rate sooner.

Sharding: data-parallel over batch — 8 batches per core, no collectives.
"""

import numpy as np

B, T, V, F = 64, 2, 512, 512
NCORES = 8
BPC = B // NCORES  # batches per core
NBLK = V // 128  # 4 row-blocks

# packed segment layout: row-block mi -> (offset, n_cols); order mi0|mi1|mi3|mi2
# keeps every segment inside one 2 KiB PSUM bank (512 fp32 columns).
SEG_OFF = {0: 0, 1: 512, 3: 896, 2: 1024}
PACK = 1280  # used columns; psum tile is 1536 (3 banks)

_NC = None


def _build_nc():
    import concourse.mybir as mybir
    import concourse.tile as tile
    from concourse import bacc

    f32 = mybir.dt.float32
    f16 = mybir.dt.float16
    COPY = mybir.ActivationFunctionType.Copy

    nc = bacc.Bacc(target_bir_lowering=False)
    yin = nc.declare_dram_parameter("yin", [BPC, 128, NBLK * V], f16, isOutput=False)
    out = nc.declare_dram_parameter("out", [BPC, 128, PACK], f16, isOutput=True)

    with tile.TileContext(nc) as tc:
        with (
            tc.tile_pool(name="inp", bufs=3) as inp_pool,
            tc.tile_pool(name="warm", bufs=1) as warm_pool,
            tc.tile_pool(name="psum", bufs=2, space="PSUM") as psum_pool,
            tc.tile_pool(name="psw", bufs=1, space="PSUM") as psw_pool,
            tc.tile_pool(name="outp", bufs=3) as out_pool,
        ):
            # PE warmup: zero matmuls overlapping the first input DMA so the
            # HAM activity window sees a busy PE from t~0.
            wz = warm_pool.tile([128, V], f16)
            nc.vector.memset(wz, 0)
            wps = psw_pool.tile([128, V], f32)
            for _ in range(4):
                nc.tensor.matmul(wps, lhsT=wz[:, :128], rhs=wz, start=True, stop=True)

            for b in range(BPC):
                it = inp_pool.tile([128, NBLK * V], f16, tag="in")
                nc.sync.dma_start(out=it, in_=yin[b])
                ps = psum_pool.tile([128, 1536], f32, tag="ps")
                for mi in (0, 1, 3, 2):
                    off = SEG_OFF[mi]
                    n_cols = V - 128 * mi
                    for ki in range(NBLK):
                        base = ki * V + mi * 128
                        nc.tensor.matmul(
                            ps[:, off : off + n_cols],
                            lhsT=it[:, base : base + 128],
                            rhs=it[:, base : ki * V + V],
                            start=(ki == 0),
                            stop=(ki == NBLK - 1),
                        )
                ot = out_pool.tile([128, PACK], f16, tag="ot")
                nc.scalar.activation(out=ot[:, 0:512], in_=ps[:, 0:512], func=COPY)
                nc.vector.tensor_copy(out=ot[:, 512:PACK], in_=ps[:, 512:PACK])
                nc.scalar.dma_start(out=out[b], in_=ot)
    if not nc.is_finalized():
        nc.finalize()
    return nc


def _get_nc():
    global _NC
    if _NC is None:
        _NC = _build_nc()
    return _NC


def _prep_shards(x: np.ndarray) -> np.ndarray:
    x = np.ascontiguousarray(np.asarray(x, dtype=np.float32))
    x0 = x[:, 0]  # (B, V, F)
    ss = np.einsum("bvf,bvf->v", x0, x0, optimize=True)
    inv_n = (1.0 / np.sqrt(ss)).astype(np.float32)
    y = x0 * inv_n[None, :, None]
    # yT[b] is (F, V); lay out as [128, 4*V] with chunk k = rows k*128..
    # at columns k*V.. so each batch is one contiguous 512 KB DMA.
    yT = np.transpose(y, (0, 2, 1)).reshape(B, NBLK, 128, V)
    return np.ascontiguousarray(np.transpose(yT, (0, 2, 1, 3))).astype(
        np.float16
    ).reshape(B, 128, NBLK * V)


def kernel(x: np.ndarray, _trace: bool = False, _trace_out: list | None = None):
    from concourse.bass_utils import run_bass_kernel_spmd

    yin = _prep_shards(x)
    nc = _get_nc()
    in_maps = [{"yin": yin[c * BPC : (c + 1) * BPC]} for c in range(NCORES)]
    res = run_bass_kernel_spmd(
        nc, in_maps, core_ids=list(range(NCORES)), trace=_trace
    )
    if _trace_out is not None:
        _trace_out.append(res)
    packed = np.concatenate(
        [np.asarray(res.results[c]["out"]) for c in range(NCORES)], axis=0
    )  # (B, 128, 1280) fp16
    full = np.empty((B, V, V), dtype=np.float32)
    for mi in range(NBLK):
        off = SEG_OFF[mi]
        n_cols = V - 128 * mi
        full[:, mi * 128 : (mi + 1) * 128, mi * 128 :] = packed[
            :, :, off : off + n_cols
        ]
    # device wrote only the upper block-triangle; mirror it down
    for mi in range(NBLK):
        for nj in range(mi + 1, NBLK):
            full[:, nj * 128 : (nj + 1) * 128, mi * 128 : (mi + 1) * 128] = (
                np.swapaxes(
                    full[:, mi * 128 : (mi + 1) * 128, nj * 128 : (nj + 1) * 128],
                    1,
                    2,
                )
            )
    return full


# revision 3
# speedup vs baseline: 1.4262x; 1.1567x over previous
"""Batched normalized-gram kernel for 8 TRN2 NeuronCores.

reference:  x (64, 2, 512, 512) fp32
    x0 = x[:, 0]                               (B=64, V=512, F=512)
    n  = sqrt(sum(x0^2, axis=(0, 2)))          (V,)
    out[b] = (x0[b] @ x0[b].T) / outer(n, n)   (B, V, V)

gram[b,i,j]/(n_i n_j) == (x0[b,i,:]/n_i) . (x0[b,j,:]/n_j), so the host
prescales rows by 1/n once and the device work is a pure batched symmetric
matmul out[b] = y[b] @ y[b].T.

Device-side structure (per core, 8 batches):
  * operands shipped as fp16 — halves input DMA, full-rate PE, fp32 PSUM
    accumulation keeps rel err ~2e-4.
  * upper block-triangle only (row-block mi covers columns mi*128..511);
    host mirrors the lower blocks.  -37.5% output DMA / PE work.
  * ONE input DMA per batch: host packs y[b].T into [128, 4*512] so the
    512 KB transfer is fully contiguous (chunk k at columns k*512..).
  * ONE output DMA per batch: the four triangle segments are packed
    [mi0|mi1|mi3|mi2] = 1280 columns so each segment's matmul stays inside
    a single 2 KB PSUM bank, and the fp16 [128, 1280] result is contiguous.
  * PSUM->SBUF copy split across Scalar (bank 0) and Vector (banks 1-2),
    which may touch PSUM concurrently on different banks.
  * a short zero-matmul warmup burst keeps the PE busy during the first
    input DMA so the HAM clock-gate reaches full rate sooner.

Sharding: data-parallel over batch — 8 batches per core, no collectives.
"""

import numpy as np

B, T, V, F = 64, 2, 512, 512
NCORES = 8
BPC = B // NCORES  # batches per core
NBLK = V // 128  # 4 row-blocks

# packed segment layout: row-block mi -> (offset, n_cols); order mi0|mi1|mi3|mi2
# keeps every segment inside one 2 KiB PSUM bank (512 fp32 columns).
SEG_OFF = {0: 0, 1: 512, 3: 896, 2: 1024}
PACK = 1280  # used columns; psum tile is 1536 (3 banks)

_NC = None


def _build_nc():
    import concourse.mybir as mybir
    import concourse.tile as tile
    from concourse import bacc

    f32 = mybir.dt.float32
    f16 = mybir.dt.float16
    COPY = mybir.ActivationFunctionType.Copy

    nc = bacc.Bacc(target_bir_lowering=False)
    yin = nc.declare_dram_parameter("yin", [BPC, 128, NBLK * V], f16, isOutput=False)
    out = nc.declare_dram_parameter("out", [BPC, 128, PACK], f16, isOutput=True)

    with tile.TileContext(nc) as tc:
        with (
            tc.tile_pool(name="inp", bufs=7) as inp_pool,
            tc.tile_pool(name="in0", bufs=4) as in0_pool,
            tc.tile_pool(name="warm", bufs=1) as warm_pool,
            tc.tile_pool(name="psum", bufs=2, space="PSUM") as psum_pool,
            tc.tile_pool(name="psw", bufs=1, space="PSUM") as psw_pool,
            tc.tile_pool(name="outp", bufs=4) as out_pool,
        ):
            # batch 0's input split into 4 chunk DMAs so the first matmul
            # group can start as soon as the first 128 KB lands.
            chunks0 = []
            for ki in range(NBLK):
                ck = in0_pool.tile([128, V], f16, tag="ck")
                nc.sync.dma_start(out=ck, in_=yin[0, :, ki * V : (ki + 1) * V])
                chunks0.append(ck)

            # PE warmup: zero matmuls filling the preamble->first-data window
            # so the HAM clock-gate is at full rate when real work starts.
            wz = warm_pool.tile([128, V], f16)
            nc.vector.memset(wz, 0)
            wps = psw_pool.tile([128, V], f32)
            for _ in range(6):
                nc.tensor.matmul(wps, lhsT=wz[:, :128], rhs=wz, start=True, stop=True)

            tiles = [None] * BPC
            for b in range(1, BPC):
                it = inp_pool.tile([128, NBLK * V], f16, tag="in")
                nc.sync.dma_start(out=it, in_=yin[b])
                tiles[b] = it

            for b in range(BPC):
                ps = psum_pool.tile([128, 1536], f32, tag="ps")
                for mi in (0, 1, 3, 2):
                    off = SEG_OFF[mi]
                    n_cols = V - 128 * mi
                    for ki in range(NBLK):
                        if b == 0:
                            lhsT = chunks0[ki][:, mi * 128 : (mi + 1) * 128]
                            rhs = chunks0[ki][:, mi * 128 :]
                        else:
                            base = ki * V + mi * 128
                            lhsT = tiles[b][:, base : base + 128]
                            rhs = tiles[b][:, base : ki * V + V]
                        nc.tensor.matmul(
                            ps[:, off : off + n_cols],
                            lhsT=lhsT,
                            rhs=rhs,
                            start=(ki == 0),
                            stop=(ki == NBLK - 1),
                        )
                ot = out_pool.tile([128, PACK], f16, tag="ot")
                nc.scalar.activation(out=ot[:, 0:512], in_=ps[:, 0:512], func=COPY)
                nc.vector.tensor_copy(out=ot[:, 512:PACK], in_=ps[:, 512:PACK])
                nc.scalar.dma_start(out=out[b], in_=ot)
    if not nc.is_finalized():
        nc.finalize()
    return nc


def _get_nc():
    global _NC
    if _NC is None:
        _NC = _build_nc()
    return _NC


def _prep_shards(x: np.ndarray) -> np.ndarray:
    x = np.ascontiguousarray(np.asarray(x, dtype=np.float32))
    x0 = x[:, 0]  # (B, V, F)
    ss = np.einsum("bvf,bvf->v", x0, x0, optimize=True)
    inv_n = (1.0 / np.sqrt(ss)).astype(np.float32)
    y = x0 * inv_n[None, :, None]
    # yT[b] is (F, V); lay out as [128, 4*V] with chunk k = rows k*128..
    # at columns k*V.. so each batch is one contiguous 512 KB DMA.
    yT = np.transpose(y, (0, 2, 1)).reshape(B, NBLK, 128, V)
    return np.ascontiguousarray(np.transpose(yT, (0, 2, 1, 3))).astype(
        np.float16
    ).reshape(B, 128, NBLK * V)


def kernel(x: np.ndarray, _trace: bool = False, _trace_out: list | None = None):
    from concourse.bass_utils import run_bass_kernel_spmd

    yin = _prep_shards(x)
    nc = _get_nc()
    in_maps = [{"yin": yin[c * BPC : (c + 1) * BPC]} for c in range(NCORES)]
    res = run_bass_kernel_spmd(
        nc, in_maps, core_ids=list(range(NCORES)), trace=_trace
    )
    if _trace_out is not None:
        _trace_out.append(res)
    packed = np.concatenate(
        [np.asarray(res.results[c]["out"]) for c in range(NCORES)], axis=0
    )  # (B, 128, 1280) fp16
    full = np.empty((B, V, V), dtype=np.float32)
    for mi in range(NBLK):
        off = SEG_OFF[mi]
        n_cols = V - 128 * mi
        full[:, mi * 128 : (mi + 1) * 128, mi * 128 :] = packed[
            :, :, off : off + n_cols
        ]
    # device wrote only the upper block-triangle; mirror it down
    for mi in range(NBLK):
        for nj in range(mi + 1, NBLK):
            full[:, nj * 128 : (nj + 1) * 128, mi * 128 : (mi + 1) * 128] = (
                np.swapaxes(
                    full[:, mi * 128 : (mi + 1) * 128, nj * 128 : (nj + 1) * 128],
                    1,
                    2,
                )
            )
    return full


# revision 5
# speedup vs baseline: 1.4523x; 1.0182x over previous
"""Batched normalized-gram kernel for 8 TRN2 NeuronCores.

reference:  x (64, 2, 512, 512) fp32
    x0 = x[:, 0]                               (B=64, V=512, F=512)
    n  = sqrt(sum(x0^2, axis=(0, 2)))          (V,)
    out[b] = (x0[b] @ x0[b].T) / outer(n, n)   (B, V, V)

gram[b,i,j]/(n_i n_j) == (x0[b,i,:]/n_i) . (x0[b,j,:]/n_j), so the host
prescales rows by 1/n once and the device work is a pure batched symmetric
matmul out[b] = y[b] @ y[b].T.

Device-side structure (per core, 8 batches; HBM-bound "ridge": 832 KB
of HBM traffic per batch ~ 2.3 us vs 2.16 us of PE streaming):
  * operands shipped as fp16 — halves input DMA, full-rate PE, fp32 PSUM
    accumulation keeps rel err ~3e-4.
  * upper block-triangle only (row-block mi covers columns mi*128..511);
    host mirrors the lower blocks.  -37.5% output DMA / PE work.
  * ONE input DMA per batch (HWDGE trigger costs ~0.65 us of descriptor
    generation, so more/smaller DMAs throttle the input stream): host
    packs y[b].T into [128, 4*512] so the 512 KB transfer is contiguous.
  * all 8 input batches buffered in SBUF up front — input prefetch never
    waits on compute.
  * fp16 output, segments packed mi0|mi1|mi3|mi2 so every segment's
    matmul accumulates inside a single 2 KiB PSUM bank.  Three separate
    PSUM tiles (bank0: mi0, bank1: mi1+mi3, bank2: mi2) give the
    PSUM->SBUF copies per-group dependencies, and the output leaves in
    two contiguous DMAs (512 cols after the Scalar copy, 768 cols after
    the Vector copies) to shorten the drain.
  * a zero-matmul warmup burst (4xN512 + 10xN128) fills the
    preamble->first-data window so the PE HAM clock-gate is at full
    rate when real matmuls start.

Sharding: data-parallel over batch — 8 batches per core, no collectives.
"""

import numpy as np

B, T, V, F = 64, 2, 512, 512
NCORES = 8
BPC = B // NCORES  # batches per core
NBLK = V // 128  # 4 row-blocks

# packed segment layout: row-block mi -> offset; order mi0|mi1|mi3|mi2
SEG_OFF = {0: 0, 1: 512, 3: 896, 2: 1024}
PACK = 1280
SPLIT = 512  # output part A = cols 0:512, part B = cols 512:1280

_NC = None


def _build_nc():
    import concourse.mybir as mybir
    import concourse.tile as tile
    from concourse import bacc

    f32 = mybir.dt.float32
    f16 = mybir.dt.float16
    COPY = mybir.ActivationFunctionType.Copy

    nc = bacc.Bacc(target_bir_lowering=False)
    yin = nc.declare_dram_parameter("yin", [BPC, 128, NBLK * V], f16, isOutput=False)
    outA = nc.declare_dram_parameter("outA", [BPC, 128, SPLIT], f16, isOutput=True)
    outB = nc.declare_dram_parameter(
        "outB", [BPC, 128, PACK - SPLIT], f16, isOutput=True
    )

    with tile.TileContext(nc) as tc:
        with (
            tc.tile_pool(name="inp", bufs=BPC) as inp_pool,
            tc.tile_pool(name="warm", bufs=1) as warm_pool,
            tc.tile_pool(name="ps0", bufs=2, space="PSUM") as ps0_pool,
            tc.tile_pool(name="ps1", bufs=2, space="PSUM") as ps1_pool,
            tc.tile_pool(name="ps2", bufs=2, space="PSUM") as ps2_pool,
            tc.tile_pool(name="psw", bufs=1, space="PSUM") as psw_pool,
            tc.tile_pool(name="outa", bufs=4) as outa_pool,
            tc.tile_pool(name="outb", bufs=4) as outb_pool,
        ):
            tiles = []
            for b in range(BPC):
                it = inp_pool.tile([128, NBLK * V], f16, tag="in")
                nc.sync.dma_start(out=it, in_=yin[b])
                tiles.append(it)

            # PE warmup on zeros: keep the PE busy from right after the
            # preamble until batch 0's data lands (~3.5 us), ending with
            # small-N matmuls so batch 0 isn't delayed behind a long one.
            wz = warm_pool.tile([128, V], f16)
            nc.gpsimd.memset(wz, 0)
            wps = psw_pool.tile([128, V], f32)
            for _ in range(4):
                nc.tensor.matmul(wps, lhsT=wz[:, :128], rhs=wz, start=True, stop=True)
            for _ in range(10):
                nc.tensor.matmul(
                    wps[:, :128],
                    lhsT=wz[:, :128],
                    rhs=wz[:, :128],
                    start=True,
                    stop=True,
                )

            for b in range(BPC):
                it = tiles[b]
                p0 = ps0_pool.tile([128, 512], f32, tag="p0")
                p1 = ps1_pool.tile([128, 512], f32, tag="p1")
                p2 = ps2_pool.tile([128, 512], f32, tag="p2")
                ps = {0: p0, 1: p1, 2: p2}
                oa = outa_pool.tile([128, SPLIT], f16, tag="oa")
                ob = outb_pool.tile([128, PACK - SPLIT], f16, tag="ob")
                for mi in (0, 1, 3, 2):
                    bank = {0: 0, 1: 1, 3: 1, 2: 2}[mi]
                    off = SEG_OFF[mi] - (0, 512, 1024)[bank]
                    n_cols = V - 128 * mi
                    for ki in range(NBLK):
                        base = ki * V + mi * 128
                        nc.tensor.matmul(
                            ps[bank][:, off : off + n_cols],
                            lhsT=it[:, base : base + 128],
                            rhs=it[:, base : ki * V + V],
                            start=(ki == 0),
                            stop=(ki == NBLK - 1),
                        )
                    if mi == 0:
                        nc.scalar.activation(out=oa, in_=ps[0], func=COPY)
                        nc.scalar.dma_start(out=outA[b], in_=oa)
                    elif mi == 3:
                        nc.vector.tensor_copy(out=ob[:, 0:512], in_=ps[1])
                    elif mi == 2:
                        nc.vector.tensor_copy(
                            out=ob[:, 512:768], in_=ps[2][:, 0:256]
                        )
                        nc.scalar.dma_start(out=outB[b], in_=ob)
    if not nc.is_finalized():
        nc.finalize()
    return nc


def _get_nc():
    global _NC
    if _NC is None:
        _NC = _build_nc()
    return _NC


def _prep_shards(x: np.ndarray) -> np.ndarray:
    x = np.ascontiguousarray(np.asarray(x, dtype=np.float32))
    x0 = x[:, 0]  # (B, V, F)
    ss = np.einsum("bvf,bvf->v", x0, x0, optimize=True)
    inv_n = (1.0 / np.sqrt(ss)).astype(np.float32)
    y = x0 * inv_n[None, :, None]
    # yT[b] is (F, V); lay out as [128, 4*V] with chunk k = rows k*128..
    # at columns k*V.. so each batch is one contiguous 512 KB DMA.
    yT = np.transpose(y, (0, 2, 1)).reshape(B, NBLK, 128, V)
    return np.ascontiguousarray(np.transpose(yT, (0, 2, 1, 3))).astype(
        np.float16
    ).reshape(B, 128, NBLK * V)


def kernel(x: np.ndarray, _trace: bool = False, _trace_out: list | None = None):
    from concourse.bass_utils import run_bass_kernel_spmd

    yin = _prep_shards(x)
    nc = _get_nc()
    in_maps = [{"yin": yin[c * BPC : (c + 1) * BPC]} for c in range(NCORES)]
    res = run_bass_kernel_spmd(
        nc, in_maps, core_ids=list(range(NCORES)), trace=_trace
    )
    if _trace_out is not None:
        _trace_out.append(res)
    packedA = np.concatenate(
        [np.asarray(res.results[c]["outA"]) for c in range(NCORES)], axis=0
    )  # (B, 128, 512)
    packedB = np.concatenate(
        [np.asarray(res.results[c]["outB"]) for c in range(NCORES)], axis=0
    )  # (B, 128, 768)
    packed = np.concatenate([packedA, packedB], axis=2)
    full = np.empty((B, V, V), dtype=np.float32)
    for mi in range(NBLK):
        off = SEG_OFF[mi]
        n_cols = V - 128 * mi
        full[:, mi * 128 : (mi + 1) * 128, mi * 128 :] = packed[
            :, :, off : off + n_cols
        ]
    # device wrote only the upper block-triangle; mirror it down
    for mi in range(NBLK):
        for nj in range(mi + 1, NBLK):
            full[:, nj * 128 : (nj + 1) * 128, mi * 128 : (mi + 1) * 128] = (
                np.swapaxes(
                    full[:, mi * 128 : (mi + 1) * 128, nj * 128 : (nj + 1) * 128],
                    1,
                    2,
                )
            )
    return full


# revision 7
# speedup vs baseline: 1.4532x; 1.0006x over previous
"""Batched normalized-gram kernel for 8 TRN2 NeuronCores.

reference:  x (64, 2, 512, 512) fp32
    x0 = x[:, 0]                               (B=64, V=512, F=512)
    n  = sqrt(sum(x0^2, axis=(0, 2)))          (V,)
    out[b] = (x0[b] @ x0[b].T) / outer(n, n)   (B, V, V)

gram[b,i,j]/(n_i n_j) == (x0[b,i,:]/n_i) . (x0[b,j,:]/n_j), so the host
prescales rows by 1/n once and the device work is a pure batched symmetric
matmul out[b] = y[b] @ y[b].T.

Device-side structure (per core, 8 batches; HBM-bound "ridge": 832 KB
of HBM traffic per batch ~ 2.3 us vs 2.16 us of PE streaming):
  * operands shipped as fp16 — halves input DMA, full-rate PE, fp32 PSUM
    accumulation keeps rel err ~3e-4.
  * upper block-triangle only (row-block mi covers columns mi*128..511);
    host mirrors the lower blocks.  -37.5% output DMA / PE work.
  * ONE input DMA per batch (HWDGE trigger costs ~0.65 us of descriptor
    generation, so more/smaller DMAs throttle the input stream): host
    packs y[b].T into [128, 4*512] so the 512 KB transfer is contiguous.
  * all 8 input batches buffered in SBUF up front — input prefetch never
    waits on compute.
  * fp16 output, segments packed mi0|mi1|mi3|mi2 so every segment's
    matmul accumulates inside a single 2 KiB PSUM bank.  Three separate
    PSUM tiles (bank0: mi0, bank1: mi1+mi3, bank2: mi2) give the
    PSUM->SBUF copies per-group dependencies, and the output leaves in
    two contiguous DMAs (512 cols after the Scalar copy, 768 cols after
    the Vector copies) to shorten the drain.
  * a zero-matmul warmup burst (4xN512 + 10xN128) fills the
    preamble->first-data window so the PE HAM clock-gate is at full
    rate when real matmuls start.

Sharding: data-parallel over batch — 8 batches per core, no collectives.
"""

import numpy as np

B, T, V, F = 64, 2, 512, 512
NCORES = 8
BPC = B // NCORES  # batches per core
NBLK = V // 128  # 4 row-blocks

# packed segment layout: row-block mi -> offset; order mi0|mi1|mi3|mi2
SEG_OFF = {0: 0, 1: 512, 3: 896, 2: 1024}
PACK = 1280
SPLIT = 512  # output part A = cols 0:512, part B = cols 512:1280

_NC = None


def _build_nc():
    import concourse.mybir as mybir
    import concourse.tile as tile
    from concourse import bacc

    f32 = mybir.dt.float32
    f16 = mybir.dt.float16
    COPY = mybir.ActivationFunctionType.Copy

    nc = bacc.Bacc(target_bir_lowering=False)
    yin = nc.declare_dram_parameter("yin", [BPC, 128, NBLK * V], f16, isOutput=False)
    outA = nc.declare_dram_parameter("outA", [BPC, 128, SPLIT], f16, isOutput=True)
    outB = nc.declare_dram_parameter(
        "outB", [BPC, 128, PACK - SPLIT], f16, isOutput=True
    )

    with tile.TileContext(nc) as tc:
        with (
            tc.tile_pool(name="inp", bufs=BPC) as inp_pool,
            tc.tile_pool(name="warm", bufs=1) as warm_pool,
            tc.tile_pool(name="ps0", bufs=2, space="PSUM") as ps0_pool,
            tc.tile_pool(name="ps1", bufs=2, space="PSUM") as ps1_pool,
            tc.tile_pool(name="ps2", bufs=2, space="PSUM") as ps2_pool,
            tc.tile_pool(name="psw", bufs=1, space="PSUM") as psw_pool,
            tc.tile_pool(name="outa", bufs=4) as outa_pool,
            tc.tile_pool(name="outb", bufs=4) as outb_pool,
        ):
            # batch 0's input lands as two 256 KB halves so its first
            # matmul group can start ~1.3 us earlier; later batches use
            # one 512 KB DMA each (every HWDGE trigger costs ~0.65 us of
            # descriptor generation, so fewer triggers = faster stream).
            h0 = inp_pool.tile([128, 2 * V], f16, tag="h0")
            nc.sync.dma_start(out=h0, in_=yin[0, :, 0 : 2 * V])
            h1 = inp_pool.tile([128, 2 * V], f16, tag="h1")
            nc.sync.dma_start(out=h1, in_=yin[0, :, 2 * V : 4 * V])
            halves0 = (h0, h1)
            tiles = [None]
            for b in range(1, BPC):
                it = inp_pool.tile([128, NBLK * V], f16, tag="in")
                nc.sync.dma_start(out=it, in_=yin[b])
                tiles.append(it)

            # PE warmup on zeros: keep the PE busy from right after the
            # preamble until batch 0's data lands, ending with small-N
            # matmuls so batch 0 isn't delayed behind a long one.
            wz = warm_pool.tile([128, V], f16)
            nc.gpsimd.memset(wz, 0)
            wps = psw_pool.tile([128, V], f32)
            for _ in range(4):
                nc.tensor.matmul(wps, lhsT=wz[:, :128], rhs=wz, start=True, stop=True)
            for _ in range(4):
                nc.tensor.matmul(
                    wps[:, :128],
                    lhsT=wz[:, :128],
                    rhs=wz[:, :128],
                    start=True,
                    stop=True,
                )

            for b in range(BPC):
                p0 = ps0_pool.tile([128, 512], f32, tag="p0")
                p1 = ps1_pool.tile([128, 512], f32, tag="p1")
                p2 = ps2_pool.tile([128, 512], f32, tag="p2")
                ps = {0: p0, 1: p1, 2: p2}
                oa = outa_pool.tile([128, SPLIT], f16, tag="oa")
                ob = outb_pool.tile([128, PACK - SPLIT], f16, tag="ob")
                for mi in (0, 1, 3, 2):
                    bank = {0: 0, 1: 1, 3: 1, 2: 2}[mi]
                    off = SEG_OFF[mi] - (0, 512, 1024)[bank]
                    n_cols = V - 128 * mi
                    for ki in range(NBLK):
                        if b == 0:
                            src = halves0[ki // 2]
                            base = (ki % 2) * V + mi * 128
                            hi = (ki % 2) * V + V
                        else:
                            src = tiles[b]
                            base = ki * V + mi * 128
                            hi = ki * V + V
                        nc.tensor.matmul(
                            ps[bank][:, off : off + n_cols],
                            lhsT=src[:, base : base + 128],
                            rhs=src[:, base:hi],
                            start=(ki == 0),
                            stop=(ki == NBLK - 1),
                        )
                    if mi == 0:
                        nc.scalar.activation(out=oa, in_=ps[0], func=COPY)
                        nc.sync.dma_start(out=outA[b], in_=oa)
                    elif mi == 3:
                        nc.vector.tensor_copy(out=ob[:, 0:512], in_=ps[1])
                        if b == BPC - 1:
                            nc.sync.dma_start(out=outB[b, :, 0:512], in_=ob[:, 0:512])
                    elif mi == 2:
                        nc.vector.tensor_copy(
                            out=ob[:, 512:768], in_=ps[2][:, 0:256]
                        )
                        if b == BPC - 1:
                            nc.sync.dma_start(
                                out=outB[b, :, 512:768], in_=ob[:, 512:768]
                            )
                        else:
                            nc.sync.dma_start(out=outB[b], in_=ob)
    if not nc.is_finalized():
        nc.finalize()
    return nc


def _get_nc():
    global _NC
    if _NC is None:
        _NC = _build_nc()
    return _NC


def _prep_shards(x: np.ndarray) -> np.ndarray:
    x = np.ascontiguousarray(np.asarray(x, dtype=np.float32))
    x0 = x[:, 0]  # (B, V, F)
    ss = np.einsum("bvf,bvf->v", x0, x0, optimize=True)
    inv_n = (1.0 / np.sqrt(ss)).astype(np.float32)
    y = x0 * inv_n[None, :, None]
    # yT[b] is (F, V); lay out as [128, 4*V] with chunk k = rows k*128..
    # at columns k*V.. so each batch is one contiguous 512 KB DMA.
    yT = np.transpose(y, (0, 2, 1)).reshape(B, NBLK, 128, V)
    return np.ascontiguousarray(np.transpose(yT, (0, 2, 1, 3))).astype(
        np.float16
    ).reshape(B, 128, NBLK * V)


def kernel(x: np.ndarray, _trace: bool = False, _trace_out: list | None = None):
    from concourse.bass_utils import run_bass_kernel_spmd

    yin = _prep_shards(x)
    nc = _get_nc()
    in_maps = [{"yin": yin[c * BPC : (c + 1) * BPC]} for c in range(NCORES)]
    res = run_bass_kernel_spmd(
        nc, in_maps, core_ids=list(range(NCORES)), trace=_trace
    )
    if _trace_out is not None:
        _trace_out.append(res)
    packedA = np.concatenate(
        [np.asarray(res.results[c]["outA"]) for c in range(NCORES)], axis=0
    )  # (B, 128, 512)
    packedB = np.concatenate(
        [np.asarray(res.results[c]["outB"]) for c in range(NCORES)], axis=0
    )  # (B, 128, 768)
    packed = np.concatenate([packedA, packedB], axis=2)
    full = np.empty((B, V, V), dtype=np.float32)
    for mi in range(NBLK):
        off = SEG_OFF[mi]
        n_cols = V - 128 * mi
        full[:, mi * 128 : (mi + 1) * 128, mi * 128 :] = packed[
            :, :, off : off + n_cols
        ]
    # device wrote only the upper block-triangle; mirror it down
    for mi in range(NBLK):
        for nj in range(mi + 1, NBLK):
            full[:, nj * 128 : (nj + 1) * 128, mi * 128 : (mi + 1) * 128] = (
                np.swapaxes(
                    full[:, mi * 128 : (mi + 1) * 128, nj * 128 : (nj + 1) * 128],
                    1,
                    2,
                )
            )
    return full
